# revision 29
# baseline (speedup 1.0000x reference)
"""DGANet dual-GAT layer on 8 Trainium2 NeuronCores (Bass/Tile).

Math (per branch b in {n, d}):
    Wh = h @ W_b                                  [4096, 256]
    e  = leaky_relu(s1_i + s2_j, 0.2)             s1 = h@(W@a1), s2 = h@(W@a2)
    att = softmax(where(adj>0, e, -9e15), axis=-1)
    f_b = elu(att @ Wh)
Output: f_n + f_d.

Sharding: 1D row-parallel over the 4096 attention rows (512 rows/core).
Each core holds its score block transposed, P^T[j, i] (j on partitions), so
the att @ Wh contraction over j runs directly on the tensor engine with the
locally-computed full Wh[j, f] as the stationary operand.  The adjacency
mask is folded into the logits as a host-prepared additive bias
(0 or -16384, bf16): exp underflows masked entries to exactly 0, which also
makes the softmax denominator a ones-column matmul over P^T.

Matmul dtype: float32r (19-bit tf32-like operand rounding, full-rate PE
streaming) when USE_F32R, else plain fp32 (bit-exact, quarter-rate PE).
"""

from contextlib import ExitStack

import numpy as np
import ml_dtypes

import concourse.bass as bass
import concourse.bacc as bacc
import concourse.mybir as mybir
import concourse.tile as tile
from concourse import bass_utils
from concourse.masks import make_identity

N, FIN, F = 4096, 512, 256
NCORES = 8
R = N // NCORES            # 512 attention rows per core
P = 128                    # partitions
NJT = N // P               # 32 j-tiles
NKT = FIN // P             # 4 fin contraction tiles
WC = F + 2                 # rhs_aug cols: [W | W@a1 | W@a2] (even N for fp32r)
MASKB = -16384.0           # additive mask: exp(x - 16384) == 0.0 in fp32
ALPHA = 0.2

USE_F32R = True

F32 = mybir.dt.float32
F32R = mybir.dt.float32r
BF16 = mybir.dt.bfloat16
FP8 = mybir.dt.float8e5
AF = mybir.ActivationFunctionType
ALU = mybir.AluOpType
BR = ("n", "d")
DT_MM = F32R if USE_F32R else F32


def build_program(reps=None):
    """reps=None: single-shot program (grading path).  reps=K: body wrapped
    in a K-iteration hardware loop, for wall-clock HW timing by slope."""
    nc = bacc.Bacc("TRN2", target_bir_lowering=False, debug=False,
                   num_devices=NCORES)

    hT = nc.dram_tensor("ht", [FIN, N], BF16, kind="ExternalInput").ap()
    hTo = nc.dram_tensor("hto", [FIN, R], BF16, kind="ExternalInput").ap()
    WAUG = {b: nc.dram_tensor(f"waug_{b}", [FIN, WC], BF16,
                              kind="ExternalInput").ap()
            for b in BR}
    MT = {b: nc.dram_tensor(f"mt_{b}", [N, R], FP8,
                            kind="ExternalInput").ap()
          for b in BR}
    OUT = nc.dram_tensor("out", [R, F], F32, kind="ExternalOutput").ap()

    with tile.TileContext(nc) as tc:
        if reps is None:
            with ExitStack() as ctx:
                _body(ctx, nc, tc, hT, hTo, WAUG, MT, OUT)
        else:
            with tc.For_i(0, reps, 1,
                          hint_engines=(mybir.EngineType.PE,)):
                with ExitStack() as ctx:
                    _body(ctx, nc, tc, hT, hTo, WAUG, MT, OUT)
    nc.compile()
    return nc


def _body(ctx, nc, tc, hT, hTo, WAUG, MT, OUT):
    CH = 4                      # hT column chunks (DMA pipelining granularity)
    CHW = N // CH               # 1024 cols per chunk

    consts = ctx.enter_context(tc.tile_pool(name="consts", bufs=1))
    # All pp_work tiles share one tag: 4 PSUM banks, recycled.
    pp_work = ctx.enter_context(tc.tile_pool(name="pp_work", bufs=5,
                                             space="PSUM"))
    # 3 single-buf accumulators shared across the (sequential) branches:
    # 3 PSUM banks, 7 total with pp_work.
    pp_acc = ctx.enter_context(tc.tile_pool(name="pp_acc", bufs=1,
                                            space="PSUM"))
    whp = ctx.enter_context(tc.tile_pool(name="whp", bufs=6))
    maskp = ctx.enter_context(tc.tile_pool(name="maskp", bufs=4))
    workp = ctx.enter_context(tc.tile_pool(name="workp", bufs=4))
    pexp = ctx.enter_context(tc.tile_pool(name="pexp", bufs=4))
    epip = ctx.enter_context(tc.tile_pool(name="epip", bufs=2))

    ident = consts.tile([P, P], F32, tag="ident")
    make_identity(nc, ident)
    onesf = consts.tile([P, P], F32, tag="onesf")
    nc.vector.memset(onesf, 1.0)
    ones_mat = consts.tile([P, P], DT_MM, tag="ones_mat")
    nc.vector.tensor_copy(out=ones_mat, in_=onesf)

    # PE warm-up: ~5us of junk matmuls on resident constants so the HAM
    # clock gate reaches 2.4 GHz while the hT DMAs are still streaming.
    wps = pp_work.tile([P, P], F32, tag="pswork", name="wps")
    for _ in range(8):
        nc.tensor.matmul(wps, lhsT=onesf, rhs=onesf, start=True, stop=True)

    # ---- stage 0: host-packed [W | W@a1 | W@a2] straight into SBUF --------
    rhs_aug = {}
    for b in BR:
        for k in range(NKT):
            t = consts.tile([P, WC], BF16, tag=f"aug_{b}{k}")
            nc.sync.dma_start(out=t, in_=WAUG[b][k * P:(k + 1) * P, :])
            rhs_aug[b, k] = t

    # wa1 chunk k replicated across 128 cols: stationary operand whose
    # matmul output is s1 already broadcast over partitions.
    wa_r = {}
    for b in BR:
        reps = []
        for k in range(NKT):
            r = consts.tile([P, P], BF16, tag=f"war_{b}{k}", name=f"war{k}")
            nc.vector.tensor_copy(
                out=r, in_=rhs_aug[b, k][:, F:F + 1].broadcast_to((P, P)))
            reps.append(r)
        wa_r[b] = reps

    # ---- own-row h block + s1 row vectors ---------------------------------
    hto_sb = []
    for k in range(NKT):
        t = consts.tile([P, R], BF16, tag=f"hto{k}")
        nc.sync.dma_start(out=t, in_=hTo[k * P:(k + 1) * P, :])
        hto_sb.append(t)

    s1b = {}
    for b in BR:
        ps1 = pp_work.tile([P, R], F32, tag="pswork")
        for k in range(NKT):
            nc.tensor.matmul(
                ps1, lhsT=wa_r[b][k], rhs=hto_sb[k],
                start=(k == 0), stop=(k == NKT - 1))
        t = consts.tile([P, R], F32, tag=f"s1b_{b}")
        nc.scalar.copy(out=t, in_=ps1)
        s1b[b] = t

    # ---- fused main loop: Wh tiles then their attention work, per pair ----
    ht_sb = {}
    for ch in range(CH):
        for k in range(NKT):
            t = consts.tile([P, CHW], BF16, tag=f"ht{k}_{ch}")
            nc.sync.dma_start(
                out=t, in_=hT[k * P:(k + 1) * P, ch * CHW:(ch + 1) * CHW])
            ht_sb[k, ch] = t

    acc = {}
    for key in (0, 1, "rs"):
        acc[key] = pp_acc.tile([P, R], F32, tag=f"acc_{key}",
                               name=f"acc_{key}")

    NPR = NJT // 2              # j-tile pairs per branch
    DELAY = 2                   # pairs of produce/consume software pipelining
    tb = {"n": [None, None], "d": [None, None]}   # elu(o)+1 per (branch, fh)
    for b in BR:

        def produce(tp):
            jt0 = 2 * tp
            ch, off = divmod(jt0 * P, CHW)
            whs, s2s = [], []
            for half in range(2):
                ps = pp_work.tile([P, WC], F32, tag="pswork", name="ps")
                for k in range(NKT):
                    nc.tensor.matmul(
                        ps, lhsT=ht_sb[k, ch][:, off + half * P:
                                              off + (half + 1) * P],
                        rhs=rhs_aug[b, k],
                        start=(k == 0), stop=(k == NKT - 1))
                wh = whp.tile([P, F], DT_MM, tag="wh", name="wh")
                s2 = whp.tile([P, 1], F32, tag="s2", name="s2", bufs=6)
                if (tp + half) % 2 == 0:
                    nc.scalar.copy(out=wh, in_=ps[:, 0:F])
                    nc.vector.tensor_copy(out=s2, in_=ps[:, F + 1:F + 2])
                else:
                    nc.vector.tensor_copy(out=wh, in_=ps[:, 0:F])
                    nc.scalar.copy(out=s2, in_=ps[:, F + 1:F + 2])
                whs.append(wh)
                s2s.append(s2)

            m = maskp.tile([P, 2 * R], FP8, tag="mask", name="m")
            nc.sync.dma_start(
                out=m.rearrange("p (two r) -> p two r", two=2),
                in_=MT[b][jt0 * P:(jt0 + 2) * P, :].rearrange(
                    "(two p) r -> p two r", two=2))
            # masked logits u = s1 + s2 + maskbias, one [P, R] op per half
            ump = workp.tile([P, 2 * R], F32, tag="ump", name="ump", bufs=4)
            for half in range(2):
                dst = ump[:, half * R:(half + 1) * R]
                msk = m[:, half * R:(half + 1) * R]
                nc.vector.scalar_tensor_tensor(
                    out=dst, in0=s1b[b], scalar=s2s[half], in1=msk,
                    op0=ALU.add, op1=ALU.add)
            # leaky_relu on the whole pair (masked entries stay <= -3276)
            lrp = workp.tile([P, 2 * R], F32, tag="lrp", name="lrp", bufs=3)
            if tp % 2 == 0:
                nc.scalar.activation(out=lrp, in_=ump, func=AF.Prelu,
                                     alpha=ALPHA)
            else:
                nc.vector.scalar_tensor_tensor(
                    out=lrp, in0=ump, scalar=ALPHA, in1=ump,
                    op0=ALU.mult, op1=ALU.max)
            ptp = pexp.tile([P, 2 * R], DT_MM, tag="ptp", name="ptp")
            nc.scalar.activation(out=ptp, in_=lrp, func=AF.Exp)
            return whs, ptp

        def consume(tp, whs, ptp):
            first, last = (tp == 0), (tp == NPR - 1)
            for half in range(2):
                pt = ptp[:, half * R:(half + 1) * R]
                st = first and half == 0
                sp = last and half == 1
                nc.tensor.matmul(acc[0], lhsT=whs[half][:, 0:P], rhs=pt,
                                 start=st, stop=sp)
                nc.tensor.matmul(acc[1], lhsT=whs[half][:, P:F], rhs=pt,
                                 start=st, stop=sp)
                nc.tensor.matmul(acc["rs"], lhsT=ones_mat, rhs=pt,
                                 start=st, stop=sp)

        inflight = []
        for tp in range(NPR):
            inflight.append((tp, *produce(tp)))
            if len(inflight) > DELAY:
                consume(*inflight.pop(0))
        for item in inflight:
            consume(*item)

        # per-branch epilogue: divide by row sums, elu (acc banks then free)
        rb = epip.tile([P, R], F32, tag="rb", name="rb", bufs=1)
        nc.vector.reciprocal(out=rb, in_=acc["rs"])
        for fh in range(2):
            o = epip.tile([P, R], F32, tag="o", name="o")
            nc.vector.scalar_tensor_tensor(
                out=o, in0=acc[fh], scalar=1.0, in1=rb,
                op0=ALU.mult, op1=ALU.mult)
            rl = epip.tile([P, R], F32, tag="rl", name="rl", bufs=1)
            nc.scalar.activation(out=rl, in_=o, func=AF.Relu)
            em = epip.tile([P, R], F32, tag="em", name="em", bufs=1)
            nc.scalar.activation(out=em, in_=o, func=AF.Exp)
            t = epip.tile([P, R], F32, tag=f"t_{b}{fh}", name="t", bufs=1)
            # t = min(exp(o), 1) + relu(o)  ==  elu(o) + 1
            nc.vector.scalar_tensor_tensor(
                out=t, in0=em, scalar=1.0, in1=rl, op0=ALU.min, op1=ALU.add)
            tb[b][fh] = t

    comb = []
    for fh in range(2):
        c = epip.tile([P, R], F32, tag="comb", name="comb")
        # c = (t_n - 2) + t_d  ==  elu(o_n) + elu(o_d)
        nc.vector.scalar_tensor_tensor(
            out=c, in0=tb["n"][fh], scalar=-2.0, in1=tb["d"][fh],
            op0=ALU.add, op1=ALU.add)
        comb.append(c)

    for it in range(R // P):
        ps = pp_work.tile([P, F], F32, tag="pswork")
        for fh in range(2):
            nc.tensor.transpose(
                ps[:, fh * P:(fh + 1) * P],
                comb[fh][:, it * P:(it + 1) * P], ident)
        o = epip.tile([P, F], F32, tag="oout")
        nc.vector.tensor_copy(out=o, in_=ps)
        nc.sync.dma_start(out=OUT[it * P:(it + 1) * P, :], in_=o)


_CACHED = None


def _get_program():
    global _CACHED
    if _CACHED is None:
        _CACHED = build_program()
    return _CACHED


def _prep_inputs(h, adj_n, adj_d, W_n, a1_n, a2_n, W_d, a1_d, a2_d):
    h = np.asarray(h, np.float32)
    hT = np.ascontiguousarray(h.T).astype(ml_dtypes.bfloat16)
    com = {"ht": hT}
    for b, W, a1, a2 in (("n", W_n, a1_n, a2_n), ("d", W_d, a1_d, a2_d)):
        W = np.asarray(W, np.float32)
        waug = np.concatenate(
            [W, W @ np.asarray(a1, np.float32),
             W @ np.asarray(a2, np.float32)], axis=1)
        com[f"waug_{b}"] = waug.astype(ml_dtypes.bfloat16)
    adj = {"n": np.asarray(adj_n), "d": np.asarray(adj_d)}
    maps = []
    for c in range(NCORES):
        m = dict(com)
        m["hto"] = np.ascontiguousarray(hT[:, c * R:(c + 1) * R])
        for b in BR:
            blk = adj[b][c * R:(c + 1) * R, :]          # [R, N]
            mt = np.where(blk.T > 0, np.float32(0.0), np.float32(MASKB))
            m[f"mt_{b}"] = mt.astype(ml_dtypes.float8_e5m2)
        maps.append(m)
    return maps


def run_on_hw(inputs, trace=False):
    nc = _get_program()
    maps = _prep_inputs(
        inputs["h"], inputs["adj_n"], inputs["adj_d"],
        inputs["W_n"], inputs["a1_n"], inputs["a2_n"],
        inputs["W_d"], inputs["a1_d"], inputs["a2_d"])
    last_err = None
    for attempt in range(3):
        try:
            res = bass_utils.run_bass_kernel_spmd(
                nc, maps, core_ids=list(range(NCORES)), trace=trace)
            break
        except Exception as e:          # transient NRT/axon failures recover
            last_err = e
            import time as _time
            _time.sleep(5)
    else:
        raise last_err
    out = np.concatenate([res.results[c]["out"] for c in range(NCORES)],
                         axis=0)
    return out, res


def kernel(**inputs):
    out, _ = run_on_hw(inputs, trace=False)
    return out



# revision 30
# speedup vs baseline: 1.0853x; 1.0853x over previous
"""DGANet dual-GAT layer on 8 Trainium2 NeuronCores (Bass/Tile).

Math (per branch b in {n, d}):
    Wh = h @ W_b                                  [4096, 256]
    e  = leaky_relu(s1_i + s2_j, 0.2)             s1 = h@(W@a1), s2 = h@(W@a2)
    att = softmax(where(adj>0, e, -9e15), axis=-1)
    f_b = elu(att @ Wh)
Output: f_n + f_d.

Sharding: 1D row-parallel over the 4096 attention rows (512 rows/core).
Each core holds its score block transposed, P^T[j, i] (j on partitions), so
the att @ Wh contraction over j runs directly on the tensor engine with the
locally-computed full Wh[j, f] as the stationary operand.  The adjacency
mask is folded into the logits as a host-prepared additive bias
(0 or -16384, bf16): exp underflows masked entries to exactly 0, which also
makes the softmax denominator a ones-column matmul over P^T.

Matmul dtype: float32r (19-bit tf32-like operand rounding, full-rate PE
streaming) when USE_F32R, else plain fp32 (bit-exact, quarter-rate PE).
"""

from contextlib import ExitStack

import numpy as np
import ml_dtypes

import concourse.bass as bass
import concourse.bacc as bacc
import concourse.mybir as mybir
import concourse.tile as tile
from concourse import bass_utils
from concourse.masks import make_identity

N, FIN, F = 4096, 512, 256
NCORES = 8
R = N // NCORES            # 512 attention rows per core
P = 128                    # partitions
NJT = N // P               # 32 j-tiles
NKT = FIN // P             # 4 fin contraction tiles
WC = F + 2                 # rhs_aug cols: [W | W@a1 | W@a2] (even N for fp32r)
MASKB = -16384.0           # additive mask: exp(x - 16384) == 0.0 in fp32
ALPHA = 0.2

USE_F32R = True

F32 = mybir.dt.float32
F32R = mybir.dt.float32r
BF16 = mybir.dt.bfloat16
FP8 = mybir.dt.float8e5
AF = mybir.ActivationFunctionType
ALU = mybir.AluOpType
BR = ("n", "d")
DT_MM = F32R if USE_F32R else F32


def build_program(reps=None):
    """reps=None: single-shot program (grading path).  reps=K: body wrapped
    in a K-iteration hardware loop, for wall-clock HW timing by slope."""
    nc = bacc.Bacc("TRN2", target_bir_lowering=False, debug=False,
                   num_devices=NCORES)

    hT = nc.dram_tensor("ht", [FIN, N], BF16, kind="ExternalInput").ap()
    hTo = nc.dram_tensor("hto", [FIN, R], BF16, kind="ExternalInput").ap()
    WAUG = {b: nc.dram_tensor(f"waug_{b}", [FIN, WC], BF16,
                              kind="ExternalInput").ap()
            for b in BR}
    MT = {b: nc.dram_tensor(f"mt_{b}", [N, R], BF16,
                            kind="ExternalInput").ap()
          for b in BR}
    OUT = nc.dram_tensor("out", [R, F], F32, kind="ExternalOutput").ap()

    with tile.TileContext(nc) as tc:
        if reps is None:
            with ExitStack() as ctx:
                _body(ctx, nc, tc, hT, hTo, WAUG, MT, OUT)
        else:
            with tc.For_i(0, reps, 1,
                          hint_engines=(mybir.EngineType.PE,)):
                with ExitStack() as ctx:
                    _body(ctx, nc, tc, hT, hTo, WAUG, MT, OUT)
    nc.compile()
    return nc


def _body(ctx, nc, tc, hT, hTo, WAUG, MT, OUT):
    CH = 4                      # hT column chunks (DMA pipelining granularity)
    CHW = N // CH               # 1024 cols per chunk

    consts = ctx.enter_context(tc.tile_pool(name="consts", bufs=1))
    # All pp_work tiles share one tag: 4 PSUM banks, recycled.
    pp_work = ctx.enter_context(tc.tile_pool(name="pp_work", bufs=5,
                                             space="PSUM"))
    # 3 single-buf accumulators shared across the (sequential) branches:
    # 3 PSUM banks, 7 total with pp_work.
    pp_acc = ctx.enter_context(tc.tile_pool(name="pp_acc", bufs=1,
                                            space="PSUM"))
    whp = ctx.enter_context(tc.tile_pool(name="whp", bufs=6))
    maskp = ctx.enter_context(tc.tile_pool(name="maskp", bufs=4))
    workp = ctx.enter_context(tc.tile_pool(name="workp", bufs=4))
    pexp = ctx.enter_context(tc.tile_pool(name="pexp", bufs=4))
    epip = ctx.enter_context(tc.tile_pool(name="epip", bufs=2))

    ident = consts.tile([P, P], F32, tag="ident")
    make_identity(nc, ident)
    onesf = consts.tile([P, P], F32, tag="onesf")
    nc.vector.memset(onesf, 1.0)
    ones_mat = consts.tile([P, P], DT_MM, tag="ones_mat")
    nc.vector.tensor_copy(out=ones_mat, in_=onesf)

    # PE warm-up: ~5us of junk matmuls on resident constants so the HAM
    # clock gate reaches 2.4 GHz while the hT DMAs are still streaming.
    wps = pp_work.tile([P, P], F32, tag="pswork", name="wps")
    for _ in range(8):
        nc.tensor.matmul(wps, lhsT=onesf, rhs=onesf, start=True, stop=True)

    # ---- stage 0: host-packed [W | W@a1 | W@a2] straight into SBUF --------
    rhs_aug = {}
    for b in BR:
        for k in range(NKT):
            t = consts.tile([P, WC], BF16, tag=f"aug_{b}{k}")
            nc.sync.dma_start(out=t, in_=WAUG[b][k * P:(k + 1) * P, :])
            rhs_aug[b, k] = t

    # wa1 chunk k replicated across 128 cols: stationary operand whose
    # matmul output is s1 already broadcast over partitions.
    wa_r = {}
    for b in BR:
        reps = []
        for k in range(NKT):
            r = consts.tile([P, P], BF16, tag=f"war_{b}{k}", name=f"war{k}")
            nc.vector.tensor_copy(
                out=r, in_=rhs_aug[b, k][:, F:F + 1].broadcast_to((P, P)))
            reps.append(r)
        wa_r[b] = reps

    # ---- own-row h block + s1 row vectors ---------------------------------
    hto_sb = []
    for k in range(NKT):
        t = consts.tile([P, R], BF16, tag=f"hto{k}")
        nc.sync.dma_start(out=t, in_=hTo[k * P:(k + 1) * P, :])
        hto_sb.append(t)

    s1b = {}
    for b in BR:
        ps1 = pp_work.tile([P, R], F32, tag="pswork")
        for k in range(NKT):
            nc.tensor.matmul(
                ps1, lhsT=wa_r[b][k], rhs=hto_sb[k],
                start=(k == 0), stop=(k == NKT - 1))
        t = consts.tile([P, R], F32, tag=f"s1b_{b}")
        nc.scalar.copy(out=t, in_=ps1)
        s1b[b] = t

    # ---- fused main loop: Wh tiles then their attention work, per pair ----
    ht_sb = {}
    for ch in range(CH):
        for k in range(NKT):
            t = consts.tile([P, CHW], BF16, tag=f"ht{k}_{ch}")
            nc.sync.dma_start(
                out=t, in_=hT[k * P:(k + 1) * P, ch * CHW:(ch + 1) * CHW])
            ht_sb[k, ch] = t

    acc = {}
    for key in (0, 1, "rs"):
        acc[key] = pp_acc.tile([P, R], F32, tag=f"acc_{key}",
                               name=f"acc_{key}")

    NPR = NJT // 2              # j-tile pairs per branch
    DELAY = 2                   # pairs of produce/consume software pipelining
    tb = {"n": [None, None], "d": [None, None]}   # elu(o)+1 per (branch, fh)
    for b in BR:

        def produce(tp):
            jt0 = 2 * tp
            ch, off = divmod(jt0 * P, CHW)
            whs, s2s = [], []
            for half in range(2):
                ps = pp_work.tile([P, WC], F32, tag="pswork", name="ps")
                for k in range(NKT):
                    nc.tensor.matmul(
                        ps, lhsT=ht_sb[k, ch][:, off + half * P:
                                              off + (half + 1) * P],
                        rhs=rhs_aug[b, k],
                        start=(k == 0), stop=(k == NKT - 1))
                wh = whp.tile([P, F], DT_MM, tag="wh", name="wh")
                s2 = whp.tile([P, 1], F32, tag="s2", name="s2", bufs=6)
                if (tp + half) % 2 == 0:
                    nc.scalar.copy(out=wh, in_=ps[:, 0:F])
                    nc.vector.tensor_copy(out=s2, in_=ps[:, F + 1:F + 2])
                else:
                    nc.vector.tensor_copy(out=wh, in_=ps[:, 0:F])
                    nc.scalar.copy(out=s2, in_=ps[:, F + 1:F + 2])
                whs.append(wh)
                s2s.append(s2)

            m = maskp.tile([P, 2 * R], BF16, tag="mask", name="m")
            nc.sync.dma_start(
                out=m.rearrange("p (two r) -> p two r", two=2),
                in_=MT[b][jt0 * P:(jt0 + 2) * P, :].rearrange(
                    "(two p) r -> p two r", two=2))
            # masked logits u = s1 + s2 + maskbias, one [P, R] op per half
            ump = workp.tile([P, 2 * R], F32, tag="ump", name="ump", bufs=4)
            for half in range(2):
                dst = ump[:, half * R:(half + 1) * R]
                msk = m[:, half * R:(half + 1) * R]
                nc.vector.scalar_tensor_tensor(
                    out=dst, in0=s1b[b], scalar=s2s[half], in1=msk,
                    op0=ALU.add, op1=ALU.add)
            # leaky_relu on the whole pair (masked entries stay <= -3276)
            lrp = workp.tile([P, 2 * R], F32, tag="lrp", name="lrp", bufs=3)
            if tp % 2 == 0:
                nc.scalar.activation(out=lrp, in_=ump, func=AF.Prelu,
                                     alpha=ALPHA)
            else:
                nc.vector.scalar_tensor_tensor(
                    out=lrp, in0=ump, scalar=ALPHA, in1=ump,
                    op0=ALU.mult, op1=ALU.max)
            ptp = pexp.tile([P, 2 * R], DT_MM, tag="ptp", name="ptp")
            nc.scalar.activation(out=ptp, in_=lrp, func=AF.Exp)
            return whs, ptp

        def consume(tp, whs, ptp):
            first, last = (tp == 0), (tp == NPR - 1)
            for half in range(2):
                pt = ptp[:, half * R:(half + 1) * R]
                st = first and half == 0
                sp = last and half == 1
                nc.tensor.matmul(acc[0], lhsT=whs[half][:, 0:P], rhs=pt,
                                 start=st, stop=sp)
                nc.tensor.matmul(acc[1], lhsT=whs[half][:, P:F], rhs=pt,
                                 start=st, stop=sp)
                nc.tensor.matmul(acc["rs"], lhsT=ones_mat, rhs=pt,
                                 start=st, stop=sp)

        inflight = []
        for tp in range(NPR):
            inflight.append((tp, *produce(tp)))
            if len(inflight) > DELAY:
                consume(*inflight.pop(0))
        for item in inflight:
            consume(*item)

        # per-branch epilogue: divide by row sums, elu (acc banks then free)
        rb = epip.tile([P, R], F32, tag="rb", name="rb", bufs=1)
        nc.vector.reciprocal(out=rb, in_=acc["rs"])
        for fh in range(2):
            o = epip.tile([P, R], F32, tag="o", name="o")
            nc.vector.scalar_tensor_tensor(
                out=o, in0=acc[fh], scalar=1.0, in1=rb,
                op0=ALU.mult, op1=ALU.mult)
            rl = epip.tile([P, R], F32, tag="rl", name="rl", bufs=1)
            nc.scalar.activation(out=rl, in_=o, func=AF.Relu)
            em = epip.tile([P, R], F32, tag="em", name="em", bufs=1)
            nc.scalar.activation(out=em, in_=o, func=AF.Exp)
            t = epip.tile([P, R], F32, tag=f"t_{b}{fh}", name="t", bufs=1)
            # t = min(exp(o), 1) + relu(o)  ==  elu(o) + 1
            nc.vector.scalar_tensor_tensor(
                out=t, in0=em, scalar=1.0, in1=rl, op0=ALU.min, op1=ALU.add)
            tb[b][fh] = t

    comb = []
    for fh in range(2):
        c = epip.tile([P, R], F32, tag="comb", name="comb")
        # c = (t_n - 2) + t_d  ==  elu(o_n) + elu(o_d)
        nc.vector.scalar_tensor_tensor(
            out=c, in0=tb["n"][fh], scalar=-2.0, in1=tb["d"][fh],
            op0=ALU.add, op1=ALU.add)
        comb.append(c)

    for it in range(R // P):
        ps = pp_work.tile([P, F], F32, tag="pswork")
        for fh in range(2):
            nc.tensor.transpose(
                ps[:, fh * P:(fh + 1) * P],
                comb[fh][:, it * P:(it + 1) * P], ident)
        o = epip.tile([P, F], F32, tag="oout")
        nc.vector.tensor_copy(out=o, in_=ps)
        nc.sync.dma_start(out=OUT[it * P:(it + 1) * P, :], in_=o)


_CACHED = None


def _get_program():
    global _CACHED
    if _CACHED is None:
        _CACHED = build_program()
    return _CACHED


def _prep_inputs(h, adj_n, adj_d, W_n, a1_n, a2_n, W_d, a1_d, a2_d):
    h = np.asarray(h, np.float32)
    hT = np.ascontiguousarray(h.T).astype(ml_dtypes.bfloat16)
    com = {"ht": hT}
    for b, W, a1, a2 in (("n", W_n, a1_n, a2_n), ("d", W_d, a1_d, a2_d)):
        W = np.asarray(W, np.float32)
        waug = np.concatenate(
            [W, W @ np.asarray(a1, np.float32),
             W @ np.asarray(a2, np.float32)], axis=1)
        com[f"waug_{b}"] = waug.astype(ml_dtypes.bfloat16)
    adj = {"n": np.asarray(adj_n), "d": np.asarray(adj_d)}
    maps = []
    for c in range(NCORES):
        m = dict(com)
        m["hto"] = np.ascontiguousarray(hT[:, c * R:(c + 1) * R])
        for b in BR:
            blk = adj[b][c * R:(c + 1) * R, :]          # [R, N]
            mt = np.where(blk.T > 0, np.float32(0.0), np.float32(MASKB))
            m[f"mt_{b}"] = mt.astype(ml_dtypes.bfloat16)
        maps.append(m)
    return maps


def run_on_hw(inputs, trace=False):
    nc = _get_program()
    maps = _prep_inputs(
        inputs["h"], inputs["adj_n"], inputs["adj_d"],
        inputs["W_n"], inputs["a1_n"], inputs["a2_n"],
        inputs["W_d"], inputs["a1_d"], inputs["a2_d"])
    last_err = None
    for attempt in range(3):
        try:
            res = bass_utils.run_bass_kernel_spmd(
                nc, maps, core_ids=list(range(NCORES)), trace=trace)
            break
        except Exception as e:          # transient NRT/axon failures recover
            last_err = e
            import time as _time
            _time.sleep(5)
    else:
        raise last_err
    out = np.concatenate([res.results[c]["out"] for c in range(NCORES)],
                         axis=0)
    return out, res


def kernel(**inputs):
    out, _ = run_on_hw(inputs, trace=False)
    return out



# revision 32
# speedup vs baseline: 1.1316x; 1.0426x over previous
"""DGANet dual-GAT layer on 8 Trainium2 NeuronCores (Bass/Tile).

Math (per branch b in {n, d}):
    Wh = h @ W_b                                  [4096, 256]
    e  = leaky_relu(s1_i + s2_j, 0.2)             s1 = h@(W@a1), s2 = h@(W@a2)
    att = softmax(where(adj>0, e, -9e15), axis=-1)
    f_b = elu(att @ Wh)
Output: f_n + f_d.

Sharding: 1D row-parallel over the 4096 attention rows (512 rows/core).
Each core holds its score block transposed, P^T[j, i] (j on partitions), so
the att @ Wh contraction over j runs directly on the tensor engine with the
locally-computed full Wh[j, f] as the stationary operand.  The adjacency
mask is folded into the logits as a host-prepared additive bias
(0 or -16384, bf16): exp underflows masked entries to exactly 0, which also
makes the softmax denominator a ones-column matmul over P^T.

Matmul dtype: float32r (19-bit tf32-like operand rounding, full-rate PE
streaming) when USE_F32R, else plain fp32 (bit-exact, quarter-rate PE).
"""

from contextlib import ExitStack

import numpy as np
import ml_dtypes

import concourse.bass as bass
import concourse.bacc as bacc
import concourse.mybir as mybir
import concourse.tile as tile
from concourse import bass_utils
from concourse.masks import make_identity

N, FIN, F = 4096, 512, 256
NCORES = 8
R = N // NCORES            # 512 attention rows per core
P = 128                    # partitions
NJT = N // P               # 32 j-tiles
NKT = FIN // P             # 4 fin contraction tiles
WC = F + 2                 # rhs_aug cols: [W | W@a1 | W@a2] (even N for fp32r)
MASKB = -16384.0           # additive mask: exp(x - 16384) == 0.0 in fp32
ALPHA = 0.2

USE_F32R = True

F32 = mybir.dt.float32
F32R = mybir.dt.float32r
BF16 = mybir.dt.bfloat16
FP8 = mybir.dt.float8e5
AF = mybir.ActivationFunctionType
ALU = mybir.AluOpType
BR = ("n", "d")
DT_MM = F32R if USE_F32R else F32


def build_program(reps=None):
    """reps=None: single-shot program (grading path).  reps=K: body wrapped
    in a K-iteration hardware loop, for wall-clock HW timing by slope."""
    nc = bacc.Bacc("TRN2", target_bir_lowering=False, debug=False,
                   num_devices=NCORES)

    hT = nc.dram_tensor("ht", [FIN, N], F32, kind="ExternalInput").ap()
    hTo = nc.dram_tensor("hto", [FIN, R], F32, kind="ExternalInput").ap()
    W = {b: nc.dram_tensor(f"w_{b}", [FIN, F], F32, kind="ExternalInput").ap()
         for b in BR}
    WT = {b: nc.dram_tensor(f"wt_{b}", [F, FIN], F32, kind="ExternalInput").ap()
          for b in BR}
    A1 = {b: nc.dram_tensor(f"a1_{b}", [F, 1], F32, kind="ExternalInput").ap()
          for b in BR}
    A2 = {b: nc.dram_tensor(f"a2_{b}", [F, 1], F32, kind="ExternalInput").ap()
          for b in BR}
    MT = {b: nc.dram_tensor(f"mt_{b}", [N, R], FP8, kind="ExternalInput").ap()
          for b in BR}
    OUT = nc.dram_tensor("out", [R, F], F32, kind="ExternalOutput").ap()

    with tile.TileContext(nc) as tc:
        if reps is None:
            with ExitStack() as ctx:
                _body(ctx, nc, tc, hT, hTo, W, WT, A1, A2, MT, OUT)
        else:
            with tc.For_i(0, reps, 1,
                          hint_engines=(mybir.EngineType.PE,)):
                with ExitStack() as ctx:
                    _body(ctx, nc, tc, hT, hTo, W, WT, A1, A2, MT, OUT)
    nc.compile()
    return nc


def _body(ctx, nc, tc, hT, hTo, W, WT, A1, A2, MT, OUT):
    CH = 4                      # hT column chunks (DMA pipelining granularity)
    CHW = N // CH               # 1024 cols per chunk

    consts = ctx.enter_context(tc.tile_pool(name="consts", bufs=1))
    rawp = ctx.enter_context(tc.tile_pool(name="rawp", bufs=3))
    # All pp_work tiles share one tag: 4 PSUM banks, recycled.
    pp_work = ctx.enter_context(tc.tile_pool(name="pp_work", bufs=5,
                                             space="PSUM"))
    # 3 single-buf accumulators shared across the (sequential) branches:
    # 3 PSUM banks, 7 total with pp_work.
    pp_acc = ctx.enter_context(tc.tile_pool(name="pp_acc", bufs=1,
                                            space="PSUM"))
    whp = ctx.enter_context(tc.tile_pool(name="whp", bufs=6))
    maskp = ctx.enter_context(tc.tile_pool(name="maskp", bufs=6))
    workp = ctx.enter_context(tc.tile_pool(name="workp", bufs=4))
    pexp = ctx.enter_context(tc.tile_pool(name="pexp", bufs=4))
    epip = ctx.enter_context(tc.tile_pool(name="epip", bufs=2))

    ident = consts.tile([P, P], F32, tag="ident")
    make_identity(nc, ident)
    onesf = consts.tile([P, P], F32, tag="onesf")
    nc.vector.memset(onesf, 1.0)
    ones_mat = consts.tile([P, P], DT_MM, tag="ones_mat")
    nc.vector.tensor_copy(out=ones_mat, in_=onesf)

    # PE warm-up: ~5us of junk matmuls on resident constants so the HAM
    # clock gate reaches 2.4 GHz while the hT DMAs are still streaming.
    wps = pp_work.tile([P, P], F32, tag="pswork", name="wps")
    for _ in range(8):
        nc.tensor.matmul(wps, lhsT=onesf, rhs=onesf, start=True, stop=True)

    # ---- stage 0: small weights in SBUF, wa = W@a on PE -------------------
    wt_sb = {}
    a_sb = {}
    for b in BR:
        for fk in range(2):
            t = consts.tile([P, FIN], F32, tag=f"wt_{b}{fk}")
            nc.sync.dma_start(out=t, in_=WT[b][fk * P:(fk + 1) * P, :])
            wt_sb[b, fk] = t
        for fk in range(2):
            t = consts.tile([P, 2], F32, tag=f"a12_{b}{fk}")
            nc.sync.dma_start(out=t[:, 0:1],
                              in_=A1[b][fk * P:(fk + 1) * P, :])
            nc.sync.dma_start(out=t[:, 1:2],
                              in_=A2[b][fk * P:(fk + 1) * P, :])
            a_sb[b, fk] = t

    # wa[b] chunks in psum [128, 2*NKT]: cols 0..3 = W@a1, 4..7 = W@a2
    wa_sb = {}
    wa_r = {}
    for b in BR:
        ps = pp_work.tile([P, 2 * NKT], F32, tag="pswork")
        for m in range(NKT):
            for fk in range(2):
                nc.tensor.matmul(
                    ps[:, 2 * m:2 * m + 2],
                    lhsT=wt_sb[b, fk][:, m * P:(m + 1) * P],
                    rhs=a_sb[b, fk],
                    start=(fk == 0), stop=(fk == 1))
        t = consts.tile([P, 2 * NKT], F32, tag=f"wa_{b}")
        nc.vector.tensor_copy(out=t, in_=ps)
        wa_sb[b] = t
        # wa1 chunk m replicated across 128 cols: stationary operand whose
        # matmul output is s1 already broadcast over partitions.
        reps = []
        for m in range(NKT):
            r = consts.tile([P, P], DT_MM, tag=f"war_{b}{m}", name=f"war{m}")
            nc.vector.tensor_copy(
                out=r, in_=t[:, 2 * m:2 * m + 1].broadcast_to((P, P)))
            reps.append(r)
        wa_r[b] = reps

    # rhs_aug[b][k] = [W rows k*128.. | (W@a2) chunk k]  -> [128, 257]
    rhs_aug = {}
    for b in BR:
        for k in range(NKT):
            t = rawp.tile([P, WC], F32, tag="augraw")
            nc.sync.dma_start(out=t[:, 0:F], in_=W[b][k * P:(k + 1) * P, :])
            nc.vector.tensor_copy(
                out=t[:, F:F + 2], in_=wa_sb[b][:, 2 * k:2 * k + 2])
            tr = consts.tile([P, WC], DT_MM, tag=f"aug_{b}{k}")
            nc.vector.tensor_copy(out=tr, in_=t)
            rhs_aug[b, k] = tr

    # ---- own-row h block + s1 row vectors ---------------------------------
    hto_sb = []
    for k in range(NKT):
        raw = rawp.tile([P, R], F32, tag="htoraw")
        nc.sync.dma_start(out=raw, in_=hTo[k * P:(k + 1) * P, :])
        t = consts.tile([P, R], DT_MM, tag=f"hto{k}")
        nc.vector.tensor_copy(out=t, in_=raw)
        hto_sb.append(t)

    s1b = {}
    for b in BR:
        ps1 = pp_work.tile([P, R], F32, tag="pswork")
        for k in range(NKT):
            nc.tensor.matmul(
                ps1, lhsT=wa_r[b][k], rhs=hto_sb[k],
                start=(k == 0), stop=(k == NKT - 1))
        t = consts.tile([P, R], F32, tag=f"s1b_{b}")
        nc.scalar.copy(out=t, in_=ps1)
        s1b[b] = t

    # ---- fused main loop: Wh tiles then their attention work, per pair ----
    mtiles = {}

    def issue_mask(b, tp):
        jt0 = 2 * tp
        m = maskp.tile([P, 2 * R], FP8, tag="mask", name="m")
        nc.sync.dma_start(
            out=m.rearrange("p (two r) -> p two r", two=2),
            in_=MT[b][jt0 * P:(jt0 + 2) * P, :].rearrange(
                "(two p) r -> p two r", two=2))
        mtiles[b, tp] = m

    ht_sb = {}
    for ch in range(CH):
        for k in range(NKT):
            raw = rawp.tile([P, CHW], F32, tag="htraw")
            nc.sync.dma_start(
                out=raw, in_=hT[k * P:(k + 1) * P, ch * CHW:(ch + 1) * CHW])
            t = consts.tile([P, CHW], DT_MM, tag=f"ht{k}_{ch}")
            if (k + ch) % 2 == 0:
                nc.vector.tensor_copy(out=t, in_=raw)
            else:
                nc.scalar.copy(out=t, in_=raw)
            ht_sb[k, ch] = t
        if ch < 2:
            for tp in range(2 * ch, 2 * ch + 2):
                issue_mask("n", tp)

    acc = {}
    for key in (0, 1, "rs"):
        acc[key] = pp_acc.tile([P, R], F32, tag=f"acc_{key}",
                               name=f"acc_{key}")

    NPR = NJT // 2              # j-tile pairs per branch
    DELAY = 2                   # pairs of produce/consume software pipelining
    tb = {"n": [None, None], "d": [None, None]}   # elu(o)+1 per (branch, fh)
    for b in BR:

        def produce(tp):
            jt0 = 2 * tp
            ch, off = divmod(jt0 * P, CHW)
            whs, s2s = [], []
            for half in range(2):
                ps = pp_work.tile([P, WC], F32, tag="pswork", name="ps")
                for k in range(NKT):
                    nc.tensor.matmul(
                        ps, lhsT=ht_sb[k, ch][:, off + half * P:
                                              off + (half + 1) * P],
                        rhs=rhs_aug[b, k],
                        start=(k == 0), stop=(k == NKT - 1))
                wh = whp.tile([P, F], DT_MM, tag="wh", name="wh")
                s2 = whp.tile([P, 1], F32, tag="s2", name="s2", bufs=6)
                if (tp + half) % 2 == 0:
                    nc.scalar.copy(out=wh, in_=ps[:, 0:F])
                    nc.vector.tensor_copy(out=s2, in_=ps[:, F + 1:F + 2])
                else:
                    nc.vector.tensor_copy(out=wh, in_=ps[:, 0:F])
                    nc.scalar.copy(out=s2, in_=ps[:, F + 1:F + 2])
                whs.append(wh)
                s2s.append(s2)

            if (b, tp) in mtiles:
                m = mtiles.pop((b, tp))
            else:
                issue_mask(b, tp)
                m = mtiles.pop((b, tp))
            # masked logits u = s1 + s2 + maskbias, one [P, R] op per half
            ump = workp.tile([P, 2 * R], F32, tag="ump", name="ump", bufs=4)
            for half in range(2):
                dst = ump[:, half * R:(half + 1) * R]
                msk = m[:, half * R:(half + 1) * R]
                nc.vector.scalar_tensor_tensor(
                    out=dst, in0=s1b[b], scalar=s2s[half], in1=msk,
                    op0=ALU.add, op1=ALU.add)
            # leaky_relu on the whole pair (masked entries stay <= -3276)
            lrp = workp.tile([P, 2 * R], F32, tag="lrp", name="lrp", bufs=3)
            if tp % 2 == 0:
                nc.scalar.activation(out=lrp, in_=ump, func=AF.Prelu,
                                     alpha=ALPHA)
            else:
                nc.vector.scalar_tensor_tensor(
                    out=lrp, in0=ump, scalar=ALPHA, in1=ump,
                    op0=ALU.mult, op1=ALU.max)
            ptp = pexp.tile([P, 2 * R], DT_MM, tag="ptp", name="ptp")
            nc.scalar.activation(out=ptp, in_=lrp, func=AF.Exp)
            return whs, ptp

        def consume(tp, whs, ptp):
            first, last = (tp == 0), (tp == NPR - 1)
            for half in range(2):
                pt = ptp[:, half * R:(half + 1) * R]
                st = first and half == 0
                sp = last and half == 1
                nc.tensor.matmul(acc[0], lhsT=whs[half][:, 0:P], rhs=pt,
                                 start=st, stop=sp)
                nc.tensor.matmul(acc[1], lhsT=whs[half][:, P:F], rhs=pt,
                                 start=st, stop=sp)
                nc.tensor.matmul(acc["rs"], lhsT=ones_mat, rhs=pt,
                                 start=st, stop=sp)

        inflight = []
        for tp in range(NPR):
            inflight.append((tp, *produce(tp)))
            if len(inflight) > DELAY:
                consume(*inflight.pop(0))
        for item in inflight:
            consume(*item)

        # per-branch epilogue: divide by row sums, elu (acc banks then free)
        rb = epip.tile([P, R], F32, tag="rb", name="rb", bufs=1)
        nc.vector.reciprocal(out=rb, in_=acc["rs"])
        for fh in range(2):
            o = epip.tile([P, R], F32, tag="o", name="o")
            nc.vector.scalar_tensor_tensor(
                out=o, in0=acc[fh], scalar=1.0, in1=rb,
                op0=ALU.mult, op1=ALU.mult)
            rl = epip.tile([P, R], F32, tag="rl", name="rl", bufs=1)
            nc.scalar.activation(out=rl, in_=o, func=AF.Relu)
            em = epip.tile([P, R], F32, tag="em", name="em", bufs=1)
            nc.scalar.activation(out=em, in_=o, func=AF.Exp)
            t = epip.tile([P, R], F32, tag=f"t_{b}{fh}", name="t", bufs=1)
            # t = min(exp(o), 1) + relu(o)  ==  elu(o) + 1
            nc.vector.scalar_tensor_tensor(
                out=t, in0=em, scalar=1.0, in1=rl, op0=ALU.min, op1=ALU.add)
            tb[b][fh] = t

    comb = []
    for fh in range(2):
        c = epip.tile([P, R], F32, tag="comb", name="comb")
        # c = (t_n - 2) + t_d  ==  elu(o_n) + elu(o_d)
        nc.vector.scalar_tensor_tensor(
            out=c, in0=tb["n"][fh], scalar=-2.0, in1=tb["d"][fh],
            op0=ALU.add, op1=ALU.add)
        comb.append(c)

    for it in range(R // P):
        ps = pp_work.tile([P, F], F32, tag="pswork")
        for fh in range(2):
            nc.tensor.transpose(
                ps[:, fh * P:(fh + 1) * P],
                comb[fh][:, it * P:(it + 1) * P], ident)
        o = epip.tile([P, F], F32, tag="oout")
        nc.vector.tensor_copy(out=o, in_=ps)
        nc.sync.dma_start(out=OUT[it * P:(it + 1) * P, :], in_=o)


_CACHED = None


def _get_program():
    global _CACHED
    if _CACHED is None:
        _CACHED = build_program()
    return _CACHED


def _prep_inputs(h, adj_n, adj_d, W_n, a1_n, a2_n, W_d, a1_d, a2_d):
    h = np.asarray(h, np.float32)
    hT = np.ascontiguousarray(h.T)
    com = {
        "ht": hT,
        "w_n": np.asarray(W_n, np.float32),
        "w_d": np.asarray(W_d, np.float32),
        "wt_n": np.ascontiguousarray(np.asarray(W_n, np.float32).T),
        "wt_d": np.ascontiguousarray(np.asarray(W_d, np.float32).T),
        "a1_n": np.asarray(a1_n, np.float32),
        "a2_n": np.asarray(a2_n, np.float32),
        "a1_d": np.asarray(a1_d, np.float32),
        "a2_d": np.asarray(a2_d, np.float32),
    }
    adj = {"n": np.asarray(adj_n), "d": np.asarray(adj_d)}
    maps = []
    for c in range(NCORES):
        m = dict(com)
        m["hto"] = np.ascontiguousarray(hT[:, c * R:(c + 1) * R])
        for b in BR:
            blk = adj[b][c * R:(c + 1) * R, :]          # [R, N]
            mt = np.where(blk.T > 0, np.float32(0.0), np.float32(MASKB))
            m[f"mt_{b}"] = mt.astype(ml_dtypes.float8_e5m2)
        maps.append(m)
    return maps


def run_on_hw(inputs, trace=False):
    nc = _get_program()
    maps = _prep_inputs(
        inputs["h"], inputs["adj_n"], inputs["adj_d"],
        inputs["W_n"], inputs["a1_n"], inputs["a2_n"],
        inputs["W_d"], inputs["a1_d"], inputs["a2_d"])
    last_err = None
    for attempt in range(3):
        try:
            res = bass_utils.run_bass_kernel_spmd(
                nc, maps, core_ids=list(range(NCORES)), trace=trace)
            break
        except Exception as e:          # transient NRT/axon failures recover
            last_err = e
            import time as _time
            _time.sleep(5)
    else:
        raise last_err
    out = np.concatenate([res.results[c]["out"] for c in range(NCORES)],
                         axis=0)
    return out, res


def kernel(**inputs):
    out, _ = run_on_hw(inputs, trace=False)
    return out

